# revision 1
# baseline (speedup 1.0000x reference)
"""Masked attention on 8 TRN2 NeuronCores — pure data-parallel over batch.

Full inputs:  q,k,v (16,2048,128) f32, mask (16,2048,2048) bool.
Output:       (16,2048,128) f32.

Per core (2 batches): computes transposed scores S^T[k,q] = K·Q^T in bf16 on
the TensorEngine (so the AV contraction lands on the partition axis with no
on-device transpose of the big matrix), applies exp with the 1/sqrt(128)
scale fused into the ScalarEngine activation (softmax max-shift skipped:
scores ~ N(0,1), exp-safe), masks multiplicatively on the VectorEngine with
a host-side-transposed (1-mask) in bf16, then AV with a ones-column appended
to V so the softmax denominator falls out of the same matmul; normalization
is a per-partition reciprocal+scale on the small [128,129] result.
"""

import numpy as np
import ml_dtypes

B, S, D = 16, 2048, 128
N_CORES = 8
BPC = B // N_CORES  # batches per core
P = 128             # partitions
QW = 512            # q-tile width (one PSUM bank of f32)
NQI = QW // P       # 4 q-subblocks per q-tile

_NC = None
LAST_RESULT = None  # BassKernelResults of the most recent run (for profiling)


def _build_nc(bpc=BPC, s=S, repeat=1):
    import concourse.bacc as bacc
    import concourse.tile as tile
    from concourse import mybir

    BPC_, S_ = bpc, s
    KB = S_ // P        # k-blocks per batch
    NQB = S_ // QW      # q-tiles per batch
    NQG = NQB // 2      # q-tile groups: 2 q-tiles share one pass over the mask
    NPAIR = KB // 2     # exp/mask processed two k-blocks (2 PSUM banks) at a time
    PDEPTH = 3          # AV matmuls pipelined this many pairs behind exp/mask
    KCH = 8             # kT loaded in KCH column-chunks so the first QK starts early
    QCH = NQG           # qT loaded per q-tile-group

    scale = 1.0 / float(np.sqrt(D))
    bf16 = mybir.dt.bfloat16
    f32 = mybir.dt.float32

    nc = bacc.Bacc()
    qT = nc.declare_dram_parameter("qT", [BPC_, P, S_], bf16, isOutput=False)
    kT = nc.declare_dram_parameter("kT", [BPC_, P, S_], bf16, isOutput=False)
    # va host-packed as [p, kb*(D+1)]: row p holds v[kb*128+p, :]+[1] for all kb,
    # so the whole batch loads as one DMA with 4KB+ partition lines
    va = nc.declare_dram_parameter(
        "va", [BPC_, P, (S_ // P) * (D + 1)], bf16, isOutput=False
    )
    # nmt[b, k, q] = 0.0 where masked else 1.0  (host-transposed)
    nmt = nc.declare_dram_parameter("nmt", [BPC_, S_, S_], bf16, isOutput=False)
    ident = nc.declare_dram_parameter("ident", [P, P], bf16, isOutput=False)
    # output stored transposed [d, q] so the store is one big-line DMA per
    # batch; the host transposes back
    out = nc.declare_dram_parameter("out", [BPC_, D, S_], bf16, isOutput=True)

    with tile.TileContext(nc) as tc:
        with (
            tc.tile_pool(name="qk", bufs=2) as qkp,
            tc.tile_pool(name="vp", bufs=2) as vp,
            tc.tile_pool(name="mp", bufs=64) as mp,
            tc.tile_pool(name="attne", bufs=6) as attnep,
            tc.tile_pool(name="attnm", bufs=8) as attnmp,
            tc.tile_pool(name="outp", bufs=8) as outp,
            tc.tile_pool(name="outT", bufs=2) as outTp,
            tc.tile_pool(name="const", bufs=1) as constp,
            tc.tile_pool(name="rp", bufs=8) as rp,
            tc.tile_pool(name="spsum", bufs=2, space="PSUM") as spsum,
            tc.tile_pool(name="avpsum", bufs=4, space="PSUM") as avpsum,
        ):
            ident_s = constp.tile([P, P], bf16)
            nc.sync.dma_start(out=ident_s[:], in_=ident[:, :])
            # dummy exp so the activation-table load (~2.7us on HW for the
            # first ACTIVATE of a set) overlaps the initial DMAs instead of
            # delaying the first real exp
            warm = constp.tile([P, 1], f32)
            nc.vector.memset(warm[:], 0.0)
            nc.scalar.activation(
                warm[:], warm[:], mybir.ActivationFunctionType.Exp
            )
            # PE warm-up burst: fills the otherwise-idle first ~1us with
            # matmul activity so the HAM clock-gate's busy window starts
            # earlier on hardware (cold PE runs at half clock); finishes
            # before the first real QK's operands arrive
            wp = avpsum.tile([P, P], f32, name="warm_mm", tag="av")
            for _ in range(8):
                nc.tensor.matmul(
                    wp[:], lhsT=ident_s[:], rhs=ident_s[:],
                    start=True, stop=True,
                )
            for _rep in range(repeat):
              for b in range(BPC_):
                # q/k loaded in chunks (separate tiles: deps are tile-granular)
                # issued so the first QK and first mask-mul start ~1us in
                qch_w = S_ // QCH
                kch_w = S_ // KCH
                qt_ch = [
                    qkp.tile([P, qch_w], bf16, name="qt_ch", tag="qt",
                             bufs=2 * QCH)
                    for _ in range(QCH)
                ]
                kt_ch = [
                    qkp.tile([P, kch_w], bf16, name="kt_ch", tag="kt",
                             bufs=2 * KCH)
                    for _ in range(KCH)
                ]
                va_s = vp.tile([P, KB, D + 1], bf16)
                # mask tiles split by q-group so the first q-group's halves
                # all arrive before it finishes consuming them
                GW = S_ // NQG
                mk_half = [
                    [mp.tile([P, GW], bf16, name="mk_h", tag="mk")
                     for _ in range(NQG)]
                    for _ in range(KB)
                ]

                def _ldq(i):
                    nc.sync.dma_start(
                        out=qt_ch[i][:], in_=qT[b, :, i * qch_w:(i + 1) * qch_w])

                def _ldk(i):
                    nc.sync.dma_start(
                        out=kt_ch[i][:], in_=kT[b, :, i * kch_w:(i + 1) * kch_w])

                def _ldm(kb, g):
                    nc.sync.dma_start(
                        out=mk_half[kb][g][:],
                        in_=nmt[b, kb * P:(kb + 1) * P, g * GW:(g + 1) * GW])

                # issue order tuned so consumers never wait: first QK needs
                # ktc0+qth0 (~1us), qh1 needs qth1, then q-group-0 mask halves
                # stream with k-chunks/va/qt-rest interleaved at their deadlines
                _ldk(0)
                _ldq(0)
                _ldq(1)
                _ldm(0, 0)
                _ldm(1, 0)
                _ldk(1)
                _ldm(2, 0)
                _ldk(2)
                _ldm(3, 0)
                _ldk(3)
                _ldm(4, 0)
                _ldk(4)
                _ldm(5, 0)
                for i in range(2, QCH):
                    _ldq(i)
                _ldm(6, 0)
                _ldm(7, 0)
                nc.sync.dma_start(
                    out=va_s[:, :, :],
                    in_=va[b, :, :].rearrange("p (kb d) -> p kb d", d=D + 1),
                )
                for i in range(5, KCH):
                    _ldk(i)
                for kb in range(8, KB):
                    _ldm(kb, 0)
                for g in range(1, NQG):
                    for kb in range(KB):
                        _ldm(kb, g)
                for qbg in range(NQG):
                    outT_s = outTp.tile([P, 2 * QW], bf16)
                    # 8 [q,129] accumulators (2 q-tiles x 4 subblocks) packed
                    # 3 per PSUM bank; the first matmul into a bank (slot%3==0,
                    # kb==0) zeroes it via start=True, and only the last matmul
                    # into a bank carries stop=True
                    av_tri = [
                        avpsum.tile([P, 3, D + 1], f32, name="av_tri", tag="av")
                        for _ in range(3)
                    ]
                    nslot = 2 * NQI
                    av_ps = [av_tri[sl // 3][:, sl % 3, :] for sl in range(nslot)]
                    attn_tiles = [[None, None] for _ in range(NPAIR)]
                    # shallower AV pipeline on the kernel's last q-group to
                    # shorten the drain tail
                    pd = PDEPTH
                    for t in range(NPAIR + pd):
                        if t < NPAIR:
                            kb0 = 2 * t
                            for qh in range(2):
                                qb = 2 * qbg + qh
                                q0 = qb * QW
                                s_ps = spsum.tile([P, 2, QW], f32)
                                for h in range(2):
                                    kb = kb0 + h
                                    kc, ko = divmod(kb * P, kch_w)
                                    qc, qo = divmod(q0, qch_w)
                                    nc.tensor.matmul(
                                        s_ps[:, h, :],
                                        lhsT=kt_ch[kc][:, ko : ko + P],
                                        rhs=qt_ch[qc][:, qo : qo + QW],
                                        start=True,
                                        stop=True,
                                    )
                                attn_e = attnep.tile([P, 2, QW], bf16)
                                nc.scalar.activation(
                                    attn_e[:, :, :],
                                    s_ps[:, :, :],
                                    mybir.ActivationFunctionType.Exp,
                                    scale=scale,
                                )
                                attn_m = attnmp.tile([P, 2, QW], bf16)
                                qg0 = q0 - qbg * GW
                                for h in range(2):
                                    nc.vector.tensor_mul(
                                        attn_m[:, h, :],
                                        attn_e[:, h, :],
                                        mk_half[kb0 + h][qbg][:, qg0 : qg0 + QW],
                                    )
                                attn_tiles[t][qh] = attn_m
                        if t >= pd:
                            tp = t - pd
                            for qh in range(2):
                                ats = attn_tiles[tp][qh]
                                for h in range(2):
                                    kb = 2 * tp + h
                                    for qi in range(NQI):
                                        sl = qh * NQI + qi
                                        nc.tensor.matmul(
                                            av_ps[sl][:, :],
                                            lhsT=ats[:, h, qi * P : (qi + 1) * P],
                                            rhs=va_s[:, kb, :],
                                            start=(kb == 0 and sl % 3 == 0),
                                            stop=(
                                                kb == KB - 1
                                                and (sl % 3 == 2 or sl == nslot - 1)
                                            ),
                                        )
                    # on the kernel's very last q-group, split the normalize
                    # stream across DVE and the now-idle ACT to shorten the tail
                    last_g = b == BPC_ - 1 and qbg == NQG - 1
                    g0 = qbg * 2 * QW
                    for slp in range(nslot // 2):
                        t_ps = avpsum.tile([P, 2, P], bf16, name="t_ps", tag="av")
                        for i in range(2):
                            sl = 2 * slp + i
                            recip = rp.tile([P, 1], f32)
                            nc.vector.reciprocal(
                                recip[:], av_ps[sl][:, D : D + 1]
                            )
                            o_s = outp.tile([P, D], bf16)
                            if last_g and i == 1:
                                nc.scalar.activation(
                                    o_s[:],
                                    av_ps[sl][:, 0:D],
                                    mybir.ActivationFunctionType.Copy,
                                    scale=recip[:],
                                )
                            else:
                                nc.vector.tensor_scalar_mul(
                                    o_s[:], av_ps[sl][:, 0:D], recip[:]
                                )
                            nc.tensor.transpose(
                                t_ps[:, i, :], o_s[:], ident_s[:]
                            )
                        if last_g and slp % 2 == 1:
                            nc.scalar.activation(
                                outT_s[:, 2 * slp * P : (2 * slp + 2) * P],
                                t_ps[:, :, :],
                                mybir.ActivationFunctionType.Copy,
                            )
                        else:
                            nc.vector.tensor_copy(
                                outT_s[:, 2 * slp * P : (2 * slp + 2) * P],
                                t_ps[:, :, :],
                            )
                        if last_g:
                            nc.sync.dma_start(
                                out=out[
                                    b, :,
                                    g0 + 2 * slp * P : g0 + (2 * slp + 2) * P,
                                ],
                                in_=outT_s[:, 2 * slp * P : (2 * slp + 2) * P],
                            )
                    if not last_g:
                        nc.sync.dma_start(
                            out=out[b, :, g0 : g0 + 2 * QW], in_=outT_s[:]
                        )
    nc.compile()
    return nc


def kernel(q, k, v, mask, _trace=False, _trace_kwargs=None):
    global _NC, LAST_RESULT
    from concourse.bass_utils import run_bass_kernel_spmd

    if _NC is None:
        _NC = _build_nc()

    bf = ml_dtypes.bfloat16
    ones = np.ones((B, S, 1), dtype=np.float32)
    # [B, S, D+1] -> [B, P, KB*(D+1)]: row p holds [v[kb*128+p], 1] for all kb
    va_full = (
        np.concatenate([np.asarray(v, np.float32), ones], axis=2)
        .reshape(B, S // P, P, D + 1)
        .transpose(0, 2, 1, 3)
        .reshape(B, P, (S // P) * (D + 1))
        .astype(bf)
    )
    qT_full = np.ascontiguousarray(
        np.asarray(q, np.float32).transpose(0, 2, 1)
    ).astype(bf)
    kT_full = np.ascontiguousarray(
        np.asarray(k, np.float32).transpose(0, 2, 1)
    ).astype(bf)
    nmt_full = np.ascontiguousarray(
        (~np.asarray(mask, bool)).transpose(0, 2, 1)
    ).astype(bf)
    ident = np.eye(P, dtype=np.float32).astype(bf)

    in_maps = []
    for c in range(N_CORES):
        lo, hi = c * BPC, (c + 1) * BPC
        in_maps.append(
            {
                "qT": qT_full[lo:hi],
                "kT": kT_full[lo:hi],
                "va": va_full[lo:hi],
                "nmt": nmt_full[lo:hi],
                "ident": ident,
            }
        )

    kw = {}
    if _trace:
        kw["trace"] = True
        if _trace_kwargs:
            kw.update(_trace_kwargs)
    LAST_RESULT = run_bass_kernel_spmd(_NC, in_maps, list(range(N_CORES)), **kw)
    res = LAST_RESULT.results
    outT = np.concatenate(
        [np.asarray(res[c]["out"]) for c in range(N_CORES)], axis=0
    ).astype(np.float32)
    return np.ascontiguousarray(outT.transpose(0, 2, 1))



# revision 3
# speedup vs baseline: 1.0731x; 1.0731x over previous
"""Masked attention on 8 TRN2 NeuronCores — pure data-parallel over batch.

Full inputs:  q,k,v (16,2048,128) f32, mask (16,2048,2048) bool.
Output:       (16,2048,128) f32.

Per core (2 batches), per 512-q x 128-k score tile:

  scores + mask land in PSUM via TWO fp8 DoubleRow matmuls (each contracts
  2x128 rows at 0.5 cyc/out-row):
    pass1: k_hi . q_hi  +  (-240*I) . mask01     (mask folded into the matmul)
    pass2: k_lo . q_hi  +  k_hi . q_lo           (hi-lo fp8 ~ bf16 precision)
  q/k are split host-side into e4m3 hi/lo pairs; the q_lo*k_lo term is
  dropped (second order).  The two operand pairs of each DoubleRow matmul
  are addressed with strided chunk APs into one big SBUF tile, so no data
  is duplicated.

  exp is split across two engines to break the ACT throughput wall:
    ~5/8 of tiles: ScalarE  attn = exp(scale*psum)            -> bf16
    ~3/8 of tiles: VectorE  attn = bitcast_bf16(int16(A*psum+B))
  (Schraudolph-style exponential: the int16 bits of A*x+B reinterpreted as
  bf16 approximate exp(x*scale) to ~1.5% — measured end-to-end rel err
  1.3e-2 vs the 2e-2 gate.)  Masked entries were pushed down by the -240
  mask term so both paths produce ~0.

  AV accumulates [128q, 129] per q-subblock in bf16 with a ones-column in
  va giving the softmax denominator; normalize = reciprocal + scale on
  DVE/ACT into a staging tile; one DMA per 1024-q group stores the output
  in natural [q, d] layout.
"""

import numpy as np
import ml_dtypes

B, S, D = 16, 2048, 128
N_CORES = 8
BPC = B // N_CORES   # batches per core
P = 128              # partitions
QW = 512             # q-tile width (one PSUM bank of f32)
KB = S // P          # k-blocks per batch (16)
NQT = S // QW        # q-tiles per batch (4)
NQG = NQT // 2       # q-groups: 2 q-tiles per group
NPAIR = KB // 2      # k-block pairs per group pass (8)

SCALE = float(1.0 / np.sqrt(np.float32(128.0)))
A2 = float(128.0 * np.log2(np.e) * SCALE)   # Schraudolph slope
B_U = 16249.0                               # Schraudolph bias (calibrated)
C_MASK = -240.0                             # mask diag coefficient (e4m3 max)
DVE_UNITS = (1, 4, 6)                       # of every 8 (g,t,qh) units

_NC = None
LAST_RESULT = None  # BassKernelResults of the most recent run (for profiling)


def _build_nc(bpc=BPC, s=S):
    import concourse.bacc as bacc
    import concourse.tile as tile
    from concourse import mybir

    BPC_, S_ = bpc, s
    KB_ = S_ // P
    NQT_ = S_ // QW
    NQG_ = NQT_ // 2
    NPAIR_ = KB_ // 2
    PD = 3            # AV matmuls pipelined this many pairs behind exp
    DA = D + 1

    bf16 = mybir.dt.bfloat16
    f32 = mybir.dt.float32
    fp8 = mybir.dt.float8e4
    i16 = mybir.dt.int16
    DR = mybir.MatmulPerfMode.DoubleRow
    MUL = mybir.AluOpType.mult
    ADD = mybir.AluOpType.add

    nc = bacc.Bacc()
    # all [d, s]-transposed fp8 hi/lo halves of q and k
    qhT = nc.declare_dram_parameter("qhT", [BPC_, P, S_], fp8, isOutput=False)
    qlT = nc.declare_dram_parameter("qlT", [BPC_, P, S_], fp8, isOutput=False)
    khT = nc.declare_dram_parameter("khT", [BPC_, P, S_], fp8, isOutput=False)
    klT = nc.declare_dram_parameter("klT", [BPC_, P, S_], fp8, isOutput=False)
    # mask01[b, k, q] = 1.0 where masked else 0.0 (host-transposed)
    m8 = nc.declare_dram_parameter("m8", [BPC_, S_, S_], fp8, isOutput=False)
    # -240 * I
    diag = nc.declare_dram_parameter("diag", [P, P], fp8, isOutput=False)
    # va host-packed as [p, kb*(D+1)]: row p holds v[kb*128+p, :]+[1] per kb
    va = nc.declare_dram_parameter(
        "va", [BPC_, P, KB_ * DA], bf16, isOutput=False
    )
    out = nc.declare_dram_parameter("out", [BPC_, S_, D], bf16, isOutput=True)

    with tile.TileContext(nc) as tc:
        with (
            tc.tile_pool(name="km", bufs=2) as kmp,
            tc.tile_pool(name="qm", bufs=2) as qmp,
            tc.tile_pool(name="vp", bufs=2) as vp,
            tc.tile_pool(name="attn", bufs=10) as attnp,
            tc.tile_pool(name="stg", bufs=3) as stgp,
            tc.tile_pool(name="rp", bufs=8) as rp,
            tc.tile_pool(name="const", bufs=1) as constp,
            tc.tile_pool(name="spsum", bufs=2, space="PSUM") as spsum,
            tc.tile_pool(name="avpsum", bufs=4, space="PSUM") as avpsum,
        ):
            # dummy exp so the activation-table load overlaps initial DMAs
            warm = constp.tile([P, 1], f32)
            nc.vector.memset(warm[:], 0.0)
            nc.scalar.activation(
                warm[:], warm[:], mybir.ActivationFunctionType.Exp
            )
            warm8 = constp.tile([P, P], fp8)
            nc.vector.memset(warm8[:], 0.0)
            # PE warm-up burst: ramps the p-state clock before real matmuls
            wp = avpsum.tile([P, P], f32, name="warm_mm", tag="av")
            for _ in range(8):
                nc.tensor.matmul(
                    wp[:], lhsT=warm8[:], rhs=warm8[:], start=True, stop=True
                )
            for b in range(BPC_):
                # KM chunks(128): [0..15]=klT [16..31]=khT [32]=diag
                km_s = kmp.tile([P, 33 * P], fp8)
                # QM chunks(512): [0..3]=qhT [4..7]=qlT [8+4kb ..]=mask kb
                qm_s = qmp.tile([P, (8 + 4 * KB_) * QW], fp8)
                va_s = vp.tile([P, KB_, DA], bf16)

                def _ldm(kb):
                    nc.sync.dma_start(
                        out=qm_s[:, (8 + 4 * kb) * QW : (12 + 4 * kb) * QW],
                        in_=m8[b, kb * P : (kb + 1) * P, :],
                    )

                # issue order tuned so early consumers never wait long
                nc.sync.dma_start(
                    out=km_s[:, 32 * P : 33 * P], in_=diag[:, :]
                )
                nc.sync.dma_start(
                    out=km_s[:, 16 * P : 32 * P], in_=khT[b, :, :]
                )
                nc.sync.dma_start(out=qm_s[:, 0 : 4 * QW], in_=qhT[b, :, :])
                _ldm(0)
                _ldm(1)
                nc.sync.dma_start(
                    out=qm_s[:, 4 * QW : 8 * QW], in_=qlT[b, :, :]
                )
                nc.sync.dma_start(out=km_s[:, 0 : 16 * P], in_=klT[b, :, :])
                _ldm(2)
                _ldm(3)
                nc.sync.dma_start(
                    out=va_s[:, :, :],
                    in_=va[b, :, :].rearrange("p (kb d) -> p kb d", d=DA),
                )
                for kb in range(4, KB_):
                    _ldm(kb)

                km3 = km_s[:].rearrange("p (c n) -> p c n", n=P)
                qm3 = qm_s[:].rearrange("p (c n) -> p c n", n=QW)

                for g in range(NQG_):
                    av_tri = [
                        avpsum.tile([P, 3, DA], f32, name="av_tri", tag="av")
                        for _ in range(3)
                    ]
                    av_ps = [av_tri[sl // 3][:, sl % 3, :] for sl in range(8)]
                    stage = stgp.tile([P, 8, P], bf16)
                    attn_tiles = [[None, None] for _ in range(NPAIR_)]
                    for t in range(NPAIR_ + PD):
                        if t < NPAIR_:
                            for qh in range(2):
                                qx = g * 2 + qh
                                s_ps = spsum.tile([P, 2, QW], f32)
                                for h in range(2):
                                    kb = 2 * t + h
                                    # pass1: kh.qh + diag.mask
                                    st = 16 - kb
                                    sr = 8 + 4 * kb
                                    nc.tensor.matmul(
                                        s_ps[:, h, :],
                                        lhsT=km3[:, 16 + kb : 33 : st, :],
                                        rhs=qm3[:, qx : qx + sr + 1 : sr, :],
                                        start=True,
                                        stop=False,
                                        perf_mode=DR,
                                    )
                                    # pass2: kl.qh + kh.ql
                                    nc.tensor.matmul(
                                        s_ps[:, h, :],
                                        lhsT=km3[:, kb : kb + 17 : 16, :],
                                        rhs=qm3[:, qx : qx + 5 : 4, :],
                                        start=False,
                                        stop=True,
                                        perf_mode=DR,
                                    )
                                unit = (g * NPAIR_ + t) * 2 + qh
                                at = attnp.tile([P, 2, QW], bf16)
                                if (unit % 8) in DVE_UNITS:
                                    nc.vector.tensor_scalar(
                                        at[:, :, :].bitcast(i16),
                                        s_ps[:, :, :],
                                        A2,
                                        B_U,
                                        MUL,
                                        ADD,
                                    )
                                else:
                                    nc.scalar.activation(
                                        at[:, :, :],
                                        s_ps[:, :, :],
                                        mybir.ActivationFunctionType.Exp,
                                        scale=SCALE,
                                    )
                                attn_tiles[t][qh] = at
                        if t >= PD:
                            tp = t - PD
                            for qh in range(2):
                                ats = attn_tiles[tp][qh]
                                for h in range(2):
                                    kb = 2 * tp + h
                                    for qi in range(4):
                                        sl = qh * 4 + qi
                                        nc.tensor.matmul(
                                            av_ps[sl][:, :],
                                            lhsT=ats[
                                                :, h, qi * P : (qi + 1) * P
                                            ],
                                            rhs=va_s[:, kb, :],
                                            start=(kb == 0 and sl % 3 == 0),
                                            stop=(
                                                kb == KB_ - 1
                                                and (sl % 3 == 2 or sl == 7)
                                            ),
                                        )
                    # normalize into the staging tile, alternating engines
                    for sl in range(8):
                        recip = rp.tile([P, 1], f32)
                        nc.vector.reciprocal(recip[:], av_ps[sl][:, D : D + 1])
                        if sl % 2 == 1:
                            nc.scalar.activation(
                                stage[:, sl, :],
                                av_ps[sl][:, 0:D],
                                mybir.ActivationFunctionType.Copy,
                                scale=recip[:],
                            )
                        else:
                            nc.vector.tensor_scalar_mul(
                                stage[:, sl, :], av_ps[sl][:, 0:D], recip[:]
                            )
                    out_ap = out[
                        b, g * 2 * QW : (g + 1) * 2 * QW, :
                    ].rearrange("(sl q) d -> q sl d", sl=8)
                    nc.sync.dma_start(out=out_ap, in_=stage[:, :, :])
    nc.compile()
    return nc


def kernel(q, k, v, mask, _trace=False, _trace_kwargs=None):
    global _NC, LAST_RESULT
    from concourse.bass_utils import run_bass_kernel_spmd

    if _NC is None:
        _NC = _build_nc()

    bf = ml_dtypes.bfloat16
    e4 = ml_dtypes.float8_e4m3

    qT = np.ascontiguousarray(np.asarray(q, np.float32).transpose(0, 2, 1))
    kT = np.ascontiguousarray(np.asarray(k, np.float32).transpose(0, 2, 1))
    qh8 = qT.astype(e4)
    ql8 = (qT - qh8.astype(np.float32)).astype(e4)
    kh8 = kT.astype(e4)
    kl8 = (kT - kh8.astype(np.float32)).astype(e4)
    m8_full = np.ascontiguousarray(
        np.asarray(mask, bool).transpose(0, 2, 1)
    ).astype(e4)
    diag = (C_MASK * np.eye(P, dtype=np.float32)).astype(e4)
    ones = np.ones((B, S, 1), dtype=np.float32)
    va_full = (
        np.concatenate([np.asarray(v, np.float32), ones], axis=2)
        .reshape(B, S // P, P, D + 1)
        .transpose(0, 2, 1, 3)
        .reshape(B, P, (S // P) * (D + 1))
        .astype(bf)
    )

    in_maps = []
    for c in range(N_CORES):
        lo, hi = c * BPC, (c + 1) * BPC
        in_maps.append(
            {
                "qhT": qh8[lo:hi],
                "qlT": ql8[lo:hi],
                "khT": kh8[lo:hi],
                "klT": kl8[lo:hi],
                "m8": m8_full[lo:hi],
                "diag": diag,
                "va": va_full[lo:hi],
            }
        )

    kw = {}
    if _trace:
        kw["trace"] = True
        if _trace_kwargs:
            kw.update(_trace_kwargs)
    LAST_RESULT = run_bass_kernel_spmd(_NC, in_maps, list(range(N_CORES)), **kw)
    res = LAST_RESULT.results
    outb = np.concatenate(
        [np.asarray(res[c]["out"]) for c in range(N_CORES)], axis=0
    )
    return np.ascontiguousarray(outb.astype(np.float32))
